# revision 1
# baseline (speedup 1.0000x reference)
"""BinaryLinear Trainium2 kernel: y = x @ sign(W).T + bias.

Contract: kernel(x, weight, bias) takes FULL unsharded numpy inputs
(x [32768,1024] f32, weight [1024,1024] f32, bias [1024] f32) and returns
the FULL output [32768,1024] f32.

Strategy (8 NeuronCores, data-parallel over tokens):
  - x is sharded into 8 x [4096, 1024] row shards; weight+bias replicated.
  - Per core, everything happens on-device:
      * weight prep: DMA W, PE-transpose 128x128 blocks, ACT Sign -> bf16
        wT [i, o] tiles. sign(W) in {-1,0,+1} is exactly representable in
        bf16, so the binarized matmul loses nothing from bf16 weights.
      * x is streamed in fp32, PE-transposed (i onto partitions), then split
        into x_hi = bf16(xT) (ACT cast) and x_lo = bf16(xT - x_hi) (DVE sub).
        x == x_hi + x_lo to ~bf16(lo) precision (rel ~2^-17), so
        y = x_hi @ s + x_lo @ s accumulated in fp32 PSUM is ~fp32-accurate
        while running the PE at full bf16 rate (fp32 matmul is 4x slower).
      * 16 bf16 matmuls (8 K-chunks x {hi,lo}) accumulate each [128, 512]
        PSUM tile; DVE adds the (DMA-broadcast) bias on eviction.
  - Emission is software-pipelined one macro-tile deep so PE never waits on
    the DVE splits of the tile it is about to multiply.
"""

import numpy as np

import concourse.bass as bass  # noqa: F401  (bass types used via bacc)
import concourse.mybir as mybir
import concourse.tile as tile
from concourse import bacc
from concourse.bass_utils import run_bass_kernel_spmd
from concourse.masks import make_identity

P = 128
N_CORES = 8
F32 = mybir.dt.float32
BF16 = mybir.dt.bfloat16


def build_kernel(
    ntok: int,
    d: int,
    o: int,
    macro: int = 512,
    two_pass: bool = True,
    lo_fp8: bool = True,
):
    """Build the per-core Bass program for x [ntok, d] f32 -> y [ntok, o] f32."""
    assert ntok % macro == 0 and macro % P == 0 and d % P == 0 and o % P == 0
    NS = macro // P  # token subtiles per macro tile
    NM = ntok // macro  # macro tiles
    IC = d // P  # contraction chunks
    OC = o // P  # output-feature 128-blocks (weight prep granularity)
    OGW = min(512, o)  # matmul free dim / psum bank width
    NOG = o // OGW
    WG = min(4, OC)  # weight-prep transpose blocks per psum tile
    FP8 = mybir.dt.float8e5  # lo-pass dtype: e5m2 (no denormal trouble for |lo|<=2^-9|x|)
    lo_fp8 = lo_fp8 and two_pass and IC % 2 == 0

    nc = bacc.Bacc(None, target_bir_lowering=False)

    x = nc.dram_tensor("x", [ntok, d], F32, kind="ExternalInput")
    w = nc.dram_tensor("w", [o, d], F32, kind="ExternalInput")
    bias = nc.dram_tensor("bias", [1, o], F32, kind="ExternalInput")
    y = nc.dram_tensor("y", [ntok, o], F32, kind="ExternalOutput")

    xr = x[:].rearrange("(m s p) d -> p m s d", p=P, s=NS)
    yr = y[:].rearrange("(m s p) o -> p m s o", p=P, s=NS)
    wr = w[:].rearrange("(oc p) d -> p oc d", p=P)

    with tile.TileContext(nc) as tc:
        with (
            tc.tile_pool(name="const", bufs=1) as const,
            tc.tile_pool(name="wstage", bufs=1) as wstage,
            tc.tile_pool(name="xpool", bufs=2) as xpool,
            tc.tile_pool(name="xtpool", bufs=2) as xtpool,
            tc.tile_pool(name="ypool", bufs=4) as ypool,
            tc.tile_pool(name="tpsum", bufs=2, space="PSUM") as tpsum,
            tc.tile_pool(name="ypsum", bufs=4, space="PSUM") as ypsum,
        ):
            # ---- constants ----
            ident = const.tile([P, P], F32)
            make_identity(nc, ident)
            bias_bc = const.tile([P, o], F32)
            nc.scalar.dma_start(bias_bc[:], bias[:].to_broadcast((P, o)))

            # ---- weight prep: wT[i, o] = sign(W[o, i]), bf16 (hi) + fp8 (lo) ----
            # Two contiguous row-chunks; prep per chunk so PE work overlaps the
            # second chunk's DMA. Emitted after split_section(0) (see below).
            wT = const.tile([P, IC, o], BF16)
            wT8 = const.tile([P, IC, o], FP8, name="wT8") if lo_fp8 else None
            w_sb = wstage.tile([P, OC, d], F32)

            def weight_dma(g):
                ocs = slice(g, g + WG)
                nc.sync.dma_start(w_sb[:, ocs], wr[:, ocs])

            def weight_prep(g):
                for ic in range(IC):
                    isl = slice(ic * P, (ic + 1) * P)
                    pw = tpsum.tile([P, WG * P], F32, tag="pw")
                    for j in range(WG):
                        nc.tensor.transpose(
                            pw[:, j * P : (j + 1) * P],
                            w_sb[:, g + j, isl],
                            ident[:],
                        )
                    osl = slice(g * P, (g + WG) * P)
                    nc.scalar.sign(wT[:, ic, osl], pw[:])
                    if lo_fp8:
                        nc.scalar.sign(wT8[:, ic, osl], pw[:])

            # HAM warm-up: ~4us of dummy matmuls during the startup DMA wait
            # so the first (DMA-gated) transposes run at 2.4 GHz, not 1.2.
            dummy = const.tile([P, 512], BF16, name="dummy")
            nc.gpsimd.memset(dummy[:], 0.0)
            dpsum = ypsum.tile([P, OGW], F32, tag="yp", name="ypdummy")
            for _ in range(10):
                nc.tensor.matmul(
                    dpsum[:], dummy[:, :P], dummy[:, :OGW], start=True, stop=True
                )

            # ---- main loop, software-pipelined one macro deep ----
            lo_dt = FP8 if lo_fp8 else BF16
            prev = None  # (hiT, loT) awaiting their matmul section

            def split_section(m):
                x_sb = xpool.tile([P, NS, d], F32, tag="x_sb")
                # x0 on the SWDGE queue (parallel with the weight DMA on sync);
                # later tiles go on sync BEHIND the weights so the x prefetch
                # can't starve the weight load the first matmuls wait on.
                dma = nc.gpsimd if m == 0 else nc.sync
                dma.dma_start(x_sb[:, :, : d // 2], xr[:, m, :, : d // 2])
                dma.dma_start(x_sb[:, :, d // 2 :], xr[:, m, :, d // 2 :])
                hiT = xtpool.tile([P, IC, macro], BF16, tag="hiT")
                loT = xtpool.tile([P, IC, macro], lo_dt, tag="loT")
                for ic in range(IC):
                    pt = tpsum.tile([P, macro], F32, tag="pt")
                    for s in range(NS):
                        nc.tensor.transpose(
                            pt[:, s * P : (s + 1) * P],
                            x_sb[:, s, ic * P : (ic + 1) * P],
                            ident[:],
                        )
                    # hi = bf16(xT) on ACT; lo = fp8/bf16(xT - hi) on DVE
                    nc.scalar.copy(hiT[:, ic], pt[:])
                    if two_pass:
                        nc.vector.tensor_tensor(
                            loT[:, ic], pt[:], hiT[:, ic], mybir.AluOpType.subtract
                        )
                return hiT, loT

            def mm_section(m, hiT, loT):
                for s in range(NS):
                    tok = slice(s * P, (s + 1) * P)
                    y_sb = ypool.tile([P, o], F32, tag="y_sb")
                    yps = [
                        ypsum.tile([P, OGW], F32, tag="yp", name=f"yp{og}")
                        for og in range(NOG)
                    ]
                    if two_pass and lo_fp8:
                        # all hi (bf16) matmuls for both output groups, then
                        # all lo DoubleRow matmuls as one burst: the DR
                        # LDWEIGHTS pipeline-fill (~400ns) is paid once per
                        # burst, not once per group.
                        for og in range(NOG):
                            osl = slice(og * OGW, (og + 1) * OGW)
                            for ic in range(IC):
                                nc.tensor.matmul(
                                    yps[og][:],
                                    hiT[:, ic, tok],
                                    wT[:, ic, osl],
                                    start=(ic == 0),
                                    stop=False,
                                )
                        for og in range(NOG):
                            osl = slice(og * OGW, (og + 1) * OGW)
                            for ic in range(0, IC, 2):
                                nc.tensor.matmul(
                                    yps[og][:],
                                    loT[:, ic : ic + 2, tok],
                                    wT8[:, ic : ic + 2, osl],
                                    start=False,
                                    stop=(ic == IC - 2),
                                    perf_mode=mybir.MatmulPerfMode.DoubleRow,
                                )
                    else:
                        for og in range(NOG):
                            osl = slice(og * OGW, (og + 1) * OGW)
                            for ic in range(IC):
                                nc.tensor.matmul(
                                    yps[og][:],
                                    hiT[:, ic, tok],
                                    wT[:, ic, osl],
                                    start=(ic == 0),
                                    stop=(not two_pass and ic == IC - 1),
                                )
                            if two_pass:
                                for ic in range(IC):
                                    nc.tensor.matmul(
                                        yps[og][:],
                                        loT[:, ic, tok],
                                        wT[:, ic, osl],
                                        start=False,
                                        stop=(ic == IC - 1),
                                    )
                    for og in range(NOG):
                        osl = slice(og * OGW, (og + 1) * OGW)
                        nc.vector.tensor_tensor(
                            y_sb[:, osl], yps[og][:], bias_bc[:, osl], mybir.AluOpType.add
                        )
                    nc.scalar.dma_start(yr[:, m, s], y_sb[:])

            def mm_first(hiT, loT):
                # macro 0, og-major: og=0 matmuls need only the first weight
                # chunk's prep; the second chunk's prep slots between the og
                # passes (its DMA long done), off the startup critical path.
                ysb = {
                    s: ypool.tile([P, o], F32, tag="y_sb", name=f"ysbf{s}")
                    for s in range(NS)
                }
                for og in range(NOG):
                    if og >= 1 and og * WG < OC:
                        weight_prep(og * WG)
                    osl = slice(og * OGW, (og + 1) * OGW)
                    for s in range(NS):
                        tok = slice(s * P, (s + 1) * P)
                        yp = ypsum.tile([P, OGW], F32, tag="yp", name=f"ypf{s % 2}")
                        for ic in range(IC):
                            nc.tensor.matmul(
                                yp[:],
                                hiT[:, ic, tok],
                                wT[:, ic, osl],
                                start=(ic == 0),
                                stop=(not two_pass and ic == IC - 1),
                            )
                        if two_pass and lo_fp8:
                            for ic in range(0, IC, 2):
                                nc.tensor.matmul(
                                    yp[:],
                                    loT[:, ic : ic + 2, tok],
                                    wT8[:, ic : ic + 2, osl],
                                    start=False,
                                    stop=(ic == IC - 2),
                                    perf_mode=mybir.MatmulPerfMode.DoubleRow,
                                )
                        elif two_pass:
                            for ic in range(IC):
                                nc.tensor.matmul(
                                    yp[:],
                                    loT[:, ic, tok],
                                    wT[:, ic, osl],
                                    start=False,
                                    stop=(ic == IC - 1),
                                )
                        nc.vector.tensor_tensor(
                            ysb[s][:, osl], yp[:], bias_bc[:, osl], mybir.AluOpType.add
                        )
                # any weight chunks not covered by an og pass (small configs)
                for g in range(max(1, NOG) * WG, OC, WG):
                    weight_prep(g)
                for s in range(NS):
                    nc.scalar.dma_start(yr[:, 0, s], ysb[s][:])

            for m in range(NM + 1):
                if m == 1:
                    # emit macro 0's matmuls BEFORE split(1): otherwise T(1)
                    # (gated on the x1 DMA, queued behind the 4 MiB weight
                    # load) sits ahead of MM(0) in the PE FIFO and blocks it
                    mm_first(*prev)
                if m < NM:
                    cur = split_section(m)
                if m == 0:
                    # weight DMAs issued up front (ahead of x1+ on the sync
                    # ring); first chunk prepped now, second inside mm_first
                    for g in range(0, OC, WG):
                        weight_dma(g)
                    weight_prep(0)
                if m >= 2:
                    mm_section(m - 1, *prev)
                if m < NM:
                    prev = cur

    nc.compile()
    return nc


_NC_CACHE: dict = {}


def _get_nc(ntok, d, o):
    key = (ntok, d, o)
    if key not in _NC_CACHE:
        _NC_CACHE[key] = build_kernel(ntok, d, o)
    return _NC_CACHE[key]


def kernel(x, weight, bias):
    x = np.ascontiguousarray(np.asarray(x, dtype=np.float32))
    weight = np.ascontiguousarray(np.asarray(weight, dtype=np.float32))
    bias = np.ascontiguousarray(np.asarray(bias, dtype=np.float32))
    ntok, d = x.shape
    o = weight.shape[0]
    assert ntok % N_CORES == 0
    shard = ntok // N_CORES

    nc = _get_nc(shard, d, o)
    bias2d = bias.reshape(1, o)
    in_maps = [
        {"x": x[i * shard : (i + 1) * shard], "w": weight, "bias": bias2d}
        for i in range(N_CORES)
    ]
    res = run_bass_kernel_spmd(nc, in_maps, core_ids=list(range(N_CORES)))
    return np.concatenate([r["y"] for r in res.results], axis=0)



# revision 3
# speedup vs baseline: 1.2205x; 1.2205x over previous
"""BinaryLinear Trainium2 kernel: y = x @ sign(W).T + bias.

Contract: kernel(x, weight, bias) takes FULL unsharded numpy inputs
(x [32768,1024] f32, weight [1024,1024] f32, bias [1024] f32) and returns
the FULL output [32768,1024] f32.

Strategy (8 NeuronCores, data-parallel over tokens):
  - x is sharded into 8 x [4096, 1024] row shards; weight+bias replicated.
  - Per core, everything happens on-device:
      * weight prep: DMA W (fp32), PE-transpose 128x128 blocks, ACT Sign ->
        e4m3 wT tiles. sign(W) in {-1,0,+1} is exact in e4m3.
      * x streamed fp32, ACT-cast to fp16 (xh), PE-transposed at 16-bit rate
        (1 cyc/row vs 2 for fp32) into PSUM, then split:
          hi = e4m3(xh) on ACT, lo = e4m3(xh - hi) on DVE.
        xh == hi + lo to ~e4m3(lo) precision; y = hi@s + lo@s accumulated in
        fp32 PSUM gives ~4e-3 overall rel err (limit 2e-2) while every
        matmul runs as an fp8 DoubleRow op at 2x the bf16 PE rate.
      * 16 DR matmuls (4 k-chunk pairs x {hi,lo} x 2 output groups)
        accumulate each [128, 2x512] PSUM tile; DVE adds the broadcast bias
        on eviction, writing y in fp16 (halves the output DMA traffic; the
        host upcasts to fp32).
  - Emission is software-pipelined one macro-tile deep, with x DMA issued a
    further macro ahead, so PE never waits on DMA or the ACT/DVE splits.
"""

import numpy as np

import concourse.bass as bass  # noqa: F401  (bass types used via bacc)
import concourse.mybir as mybir
import concourse.tile as tile
from concourse import bacc
from concourse.bass_utils import run_bass_kernel_spmd
from concourse.masks import make_identity

P = 128
N_CORES = 8
F32 = mybir.dt.float32
F16 = mybir.dt.float16
E4 = mybir.dt.float8e4  # e4m3
DR = mybir.MatmulPerfMode.DoubleRow


def build_kernel(ntok: int, d: int, o: int, macro: int = 512):
    """Build the per-core Bass program for x [ntok, d] f32 -> y [ntok, o] f16."""
    assert ntok % macro == 0 and macro % P == 0 and d % P == 0 and o % P == 0
    NS = macro // P  # token subtiles per macro tile
    NM = ntok // macro  # macro tiles
    IC = d // P  # contraction 128-chunks
    NPAIR = IC // 2  # DoubleRow k-chunk pairs
    OC = o // P  # output-feature 128-blocks (weight prep granularity)
    OGW = min(512, o)  # psum bank free width
    NOG = o // OGW  # output groups
    WG = min(4, OC)  # weight-prep transpose blocks per psum tile
    assert IC % 2 == 0

    nc = bacc.Bacc(None, target_bir_lowering=False)

    x = nc.dram_tensor("x", [ntok, d], F32, kind="ExternalInput")
    w = nc.dram_tensor("w", [o, d], F32, kind="ExternalInput")
    bias = nc.dram_tensor("bias", [1, o], F32, kind="ExternalInput")
    y = nc.dram_tensor("y", [ntok, o], F16, kind="ExternalOutput")

    xr = x[:].rearrange("(m s p) d -> p m s d", p=P, s=NS)
    yr = y[:].rearrange("(m s p) o -> p m s o", p=P, s=NS)
    wr = w[:].rearrange("(oc p) d -> p oc d", p=P)

    with tile.TileContext(nc) as tc:
        with (
            tc.tile_pool(name="const", bufs=1) as const,
            tc.tile_pool(name="wstage", bufs=1) as wstage,
            tc.tile_pool(name="xpool", bufs=2) as xpool,
            tc.tile_pool(name="xhpool", bufs=2) as xhpool,
            tc.tile_pool(name="xtpool", bufs=2) as xtpool,
            tc.tile_pool(name="ypool", bufs=4) as ypool,
            tc.tile_pool(name="tpsum", bufs=2, space="PSUM") as tpsum,
            tc.tile_pool(name="ypsum", bufs=2, space="PSUM") as ypsum,
        ):
            # ---- x DMA: issued up to one macro ahead of the compute ----
            def xdma(m):
                t = xpool.tile([P, NS, d], F32, tag="x_sb", name=f"x{m % 2}")
                # x0/x1 ride the SWDGE queue in parallel with the weight DMA
                # on sync; later macros go on sync behind the weights.
                dma = nc.gpsimd if m <= 1 else nc.sync
                dma.dma_start(t[:, :, : d // 2], xr[:, m, :, : d // 2])
                dma.dma_start(t[:, :, d // 2 :], xr[:, m, :, d // 2 :])
                return t

            x_tiles = {0: xdma(0)}

            def weight_dma(g):
                ocs = slice(g, g + WG)
                nc.sync.dma_start(w_sb[:, ocs], wr[:, ocs])

            w_sb = wstage.tile([P, OC, d], F32)
            weight_dma(0)
            weight_dma(WG)
            x_tiles[1] = xdma(1)

            # ---- constants ----
            ident16 = const.tile([P, P], F16, name="ident16")
            make_identity(nc, ident16)
            ident32 = const.tile([P, P], F32, name="ident32")
            make_identity(nc, ident32)
            bias_bc = const.tile([P, o], F32)
            nc.scalar.dma_start(bias_bc[:], bias[:].to_broadcast((P, o)))

            # ---- weight prep: wT[i, o] = sign(W[o, i]) in e4m3 ----
            wT = const.tile([P, IC, o], E4, name="wT")

            def weight_prep(g):
                for ic in range(IC):
                    isl = slice(ic * P, (ic + 1) * P)
                    pw = tpsum.tile([P, 2, OGW], F32, tag="pt", name="pw")
                    for j in range(WG):
                        nc.tensor.transpose(
                            pw[:, 0, j * P : (j + 1) * P],
                            w_sb[:, g + j, isl],
                            ident32[:],
                        )
                    osl = slice(g * P, (g + WG) * P)
                    nc.scalar.sign(wT[:, ic, osl], pw[:, 0])

            # HAM warm-up: ~4us of dummy matmuls during the startup DMA wait
            # so the first (DMA-gated) transposes run at 2.4 GHz, not 1.2.
            dummy = const.tile([P, 512], mybir.dt.bfloat16, name="dummy")
            nc.gpsimd.memset(dummy[:], 0.0)
            dpsum = ypsum.tile([P, 2, OGW], F32, tag="yp", name="ypdummy")
            for _ in range(10):
                nc.tensor.matmul(
                    dpsum[:, 0], dummy[:, :P], dummy[:], start=True, stop=True
                )

            # ---- per-macro sections ----
            def split_compute(m, x_sb):
                xh = xhpool.tile([P, NS, d], F16, tag="xh")
                hiT = xtpool.tile([P, IC, macro], E4, tag="hiT")
                loT = xtpool.tile([P, IC, macro], E4, tag="loT")
                for i in range(NPAIR):
                    csl = slice(2 * i * P, (2 * i + 2) * P)
                    nc.scalar.copy(xh[:, :, csl], x_sb[:, :, csl])
                    pt = tpsum.tile([P, 2, macro], F16, tag="pt")
                    for c in (0, 1):
                        ic = 2 * i + c
                        for s in range(NS):
                            nc.tensor.transpose(
                                pt[:, c, s * P : (s + 1) * P],
                                xh[:, s, ic * P : (ic + 1) * P],
                                ident16[:],
                            )
                    psl = slice(2 * i, 2 * i + 2)
                    nc.scalar.copy(hiT[:, psl], pt[:])
                    nc.vector.tensor_tensor(
                        loT[:, psl], pt[:], hiT[:, psl], mybir.AluOpType.subtract
                    )
                return hiT, loT

            def mm_tile(yp, hiT, loT, tok, osl, lead):
                # one [128, OGW] output tile: 4 hi-DR + 4 lo-DR e4m3 matmuls
                for i in range(NPAIR):
                    ksl = slice(2 * i, 2 * i + 2)
                    nc.tensor.matmul(
                        yp,
                        hiT[:, ksl, tok],
                        wT[:, ksl, osl],
                        start=(lead and i == 0),
                        stop=False,
                        perf_mode=DR,
                    )
                for i in range(NPAIR):
                    ksl = slice(2 * i, 2 * i + 2)
                    nc.tensor.matmul(
                        yp,
                        loT[:, ksl, tok],
                        wT[:, ksl, osl],
                        start=False,
                        stop=(i == NPAIR - 1),
                        perf_mode=DR,
                    )

            def mm_section(m, hiT, loT):
                for s in range(NS):
                    tok = slice(s * P, (s + 1) * P)
                    yp2 = ypsum.tile([P, 2, OGW], F32, tag="yp")
                    for og in range(NOG):
                        osl = slice(og * OGW, (og + 1) * OGW)
                        mm_tile(yp2[:, og], hiT, loT, tok, osl, lead=True)
                    y_sb = ypool.tile([P, o], F16, tag="y_sb")
                    nc.vector.tensor_tensor(
                        y_sb[:], yp2[:], bias_bc[:], mybir.AluOpType.add
                    )
                    nc.scalar.dma_start(yr[:, m, s], y_sb[:])

            def mm_first(hiT, loT):
                # macro 0, og-major: og=0 matmuls need only the first weight
                # chunk's prep; the second chunk's prep slots between the og
                # passes (its DMA long done), off the startup critical path.
                ysb = {
                    s: ypool.tile([P, o], F16, tag="y_sb", name=f"ysbf{s}")
                    for s in range(NS)
                }
                for og in range(NOG):
                    if og >= 1 and og * WG < OC:
                        weight_prep(og * WG)
                    osl = slice(og * OGW, (og + 1) * OGW)
                    for s in range(NS):
                        tok = slice(s * P, (s + 1) * P)
                        ypf = ypsum.tile(
                            [P, 2, OGW], F32, tag="yp", name=f"ypf{s % 2}"
                        )
                        mm_tile(ypf[:, 0], hiT, loT, tok, osl, lead=True)
                        nc.vector.tensor_tensor(
                            ysb[s][:, osl],
                            ypf[:, 0],
                            bias_bc[:, osl],
                            mybir.AluOpType.add,
                        )
                for g in range(max(1, NOG) * WG, OC, WG):
                    weight_prep(g)
                for s in range(NS):
                    nc.scalar.dma_start(yr[:, 0, s], ysb[s][:])

            # ---- main loop, software-pipelined one macro deep ----
            hl0 = split_compute(0, x_tiles[0])
            weight_prep(0)
            mm_first(*hl0)
            prev = None
            for m in range(1, NM):
                if m + 1 < NM:
                    x_tiles[m + 1] = xdma(m + 1)
                cur = split_compute(m, x_tiles[m])
                if m >= 2:
                    mm_section(m - 1, *prev)
                prev = cur
            mm_section(NM - 1, *prev)

    nc.compile()
    return nc


_NC_CACHE: dict = {}


def _get_nc(ntok, d, o):
    key = (ntok, d, o)
    if key not in _NC_CACHE:
        _NC_CACHE[key] = build_kernel(ntok, d, o)
    return _NC_CACHE[key]


def kernel(x, weight, bias):
    x = np.ascontiguousarray(np.asarray(x, dtype=np.float32))
    weight = np.ascontiguousarray(np.asarray(weight, dtype=np.float32))
    bias = np.ascontiguousarray(np.asarray(bias, dtype=np.float32))
    ntok, d = x.shape
    o = weight.shape[0]
    assert ntok % N_CORES == 0
    shard = ntok // N_CORES

    nc = _get_nc(shard, d, o)
    bias2d = bias.reshape(1, o)
    in_maps = [
        {"x": x[i * shard : (i + 1) * shard], "w": weight, "bias": bias2d}
        for i in range(N_CORES)
    ]
    res = run_bass_kernel_spmd(nc, in_maps, core_ids=list(range(N_CORES)))
    out = np.concatenate([np.asarray(r["y"]) for r in res.results], axis=0)
    return out.astype(np.float32)
